# revision 1
# baseline (speedup 1.0000x reference)
"""Trainium2 Bass kernel: per-row 3D histogram binning (BinDensityEncoder).

states [512, 8192, 3] f32 -> per-row joint histogram over 32^3 = 32768 bins,
normalized by N=8192.  Data-parallel over batch rows: 8 cores x 64 rows.

Algorithm (per core, per batch row):
  - bin index per dim: i = clamp(floor((x+3) * 16/3), 0, 31)   (uniform edges)
  - linear index: lin = i0 + 32*i1 + 1024*i2  in [0, 32768)
  - split lin = hi*256 + lo  (hi in [0,128), lo in [0,256))
  - one-hot(hi) [128pts, 128] and one-hot(lo) [128pts, 256] in bf16 via
    iota-compare (tensor_scalar is_equal), then TensorE matmul
    one-hot(hi)^T @ one-hot(lo) accumulated in PSUM over the row's 64
    point-chunks -> [128, 256] = the row histogram.
"""

import os
import sys

import numpy as np

for _p in ("/opt/trn_rl_repo", "/root/.axon_site/_ro/trn_rl_repo"):
    if _p not in sys.path and os.path.isdir(_p):
        sys.path.insert(0, _p)

P = 128
N_CORES = 8
B_FULL = 512
ROWS = B_FULL // N_CORES  # 64 rows per core
NPTS = 8192
QP = NPTS // P  # 64 points per partition (also: chunks per row)
W_HI = 128
W_LO = 256
OUT_DIM = 32768
SCALE = 16.0 / 3.0  # 1 / bin_width, bin_width = 6/32
ROW_BLOCK = 8  # rows per input/output DMA batch

# Fraction of Ohi/Olo one-hot builds offloaded to GPSIMD = NUM / 8
GPSIMD_NUM = int(os.environ.get("HISTO_GPSIMD_NUM", "0"))
GPSIMD_LO_NUM = int(os.environ.get("HISTO_GPSIMD_LO_NUM", "0"))
GPSIMD_DEN = 8
# Fraction of chunks whose one-hots are built on the scalar engine (ACT)
# via relu(1 - (iota - idx)^2) instead of DVE is_equal = ACT_NUM / 8.
# Measured on HW: ACT per-op overhead makes this slower (2.20ms at 3/8 vs
# 1.53ms all-DVE), so it stays off by default.
ACT_NUM = int(os.environ.get("HISTO_ACT_NUM", "0"))

_CACHE = {}

# Set by the most recent kernel() call when BASS_HISTO_TRACE=1; test.py reads it.
last_results = None


def _legalize_drain_waits(nc, max_waits=1):
    """This image's walrus allows only one sync-wait per instruction.
    Move extra waits onto inserted NoOps just before (same engine, so
    per-engine program order preserves the wait semantics)."""
    import concourse.mybir as mybir

    elide = os.environ.get("HISTO_ELIDE_SELF_WAITS", "0") == "1"
    for fn in nc.m.functions:
        for bb in fn.blocks:
            # Running count of each semaphore's increments issued by each
            # engine earlier in this block: a wait on a sem that only this
            # engine increments, with value <= increments already issued by
            # this engine's earlier (in-order-completing) instructions, is
            # guaranteed satisfied and can be dropped.
            pre = {}
            new_list = []
            changed = False
            for ins in bb.instructions:
                si = ins.sync_info
                waits = list(si.on_wait) if si else []
                if len(waits) > max_waits:
                    kept = []
                    extra = []
                    for w in waits:
                        nm = getattr(w, "ant_name", "") or ""
                        # DVE only: its per-op DRAIN guarantees the previous
                        # op (and its sem inc) completed before the next issues.
                        own = (
                            nm.rsplit("_", 1)[0] == str(ins.engine).split(".")[-1]
                            and nm.startswith("DVE")
                        )
                        if (
                            elide
                            and own
                            and w.wait_value <= pre.get((ins.engine, nm), 0)
                        ):
                            changed = True
                            continue
                        (kept if len(kept) < max_waits else extra).append(w)
                    for k, w in enumerate(extra):
                        new_list.append(
                            mybir.InstNoOp(
                                name=f"{ins.name}-w{k}",
                                engine=ins.engine,
                                ins=[],
                                outs=[],
                                sync_info=mybir.SyncInfo(on_update=[], on_wait=[w]),
                            )
                        )
                    si.on_wait = kept
                    changed = True
                if si:
                    for u in si.on_update:
                        nm = getattr(u, "ant_name", "") or ""
                        key = (ins.engine, nm)
                        pre[key] = pre.get(key, 0) + u.update_value
                new_list.append(ins)
            if changed:
                bb.instructions = new_list
    return nc


def _build_program(legalize=True):
    import concourse.bass as bass
    import concourse.mybir as mybir
    import concourse.tile as tile
    from contextlib import ExitStack

    f32 = mybir.dt.float32
    i32 = mybir.dt.int32
    bf16 = mybir.dt.bfloat16
    Alu = mybir.AluOpType
    Act = mybir.ActivationFunctionType

    nc = bass.Bass()
    states_d = nc.declare_dram_parameter("states", [ROWS, NPTS, 3], f32, isOutput=False)
    iota_d = nc.declare_dram_parameter("iota", [P, W_LO], bf16, isOutput=False)
    out_d = nc.declare_dram_parameter("out", [ROWS, OUT_DIM], f32, isOutput=True)

    with tile.TileContext(nc) as tc, ExitStack() as ctx:
        const_pool = ctx.enter_context(tc.tile_pool(name="const", bufs=1))
        pool = ctx.enter_context(tc.tile_pool(name="main", bufs=int(os.environ.get("HISTO_MAIN_BUFS", "2"))))
        opool = ctx.enter_context(tc.tile_pool(name="oh", bufs=int(os.environ.get("HISTO_OH_BUFS", "3"))))
        psum = ctx.enter_context(
            tc.tile_pool(name="psum", bufs=int(os.environ.get("HISTO_PSUM_BUFS", "2")), space="PSUM")
        )

        iota_t = const_pool.tile([P, W_LO], bf16)
        nc.sync.dma_start(iota_t[:], iota_d[:])
        # bias = 16 - 0.5: HW f32->int32 convert rounds to nearest, so compute
        # v = true_v - 0.5 and round to get floor(true_v).
        bias16 = const_pool.tile([P, 1], f32)
        nc.gpsimd.memset(bias16[:], 15.5)
        one_c = const_pool.tile([P, 1], f32)
        nc.gpsimd.memset(one_c[:], 1.0)

        n_blocks = ROWS // ROW_BLOCK
        for blk in range(n_blocks):
            # ---- load ROW_BLOCK rows: [128, ROW_BLOCK * QP * 3] f32
            st = pool.tile([P, ROW_BLOCK * QP * 3], f32, tag="st")
            nc.sync.dma_start(
                st[:].rearrange("p (r q s) -> p r q s", r=ROW_BLOCK, s=3),
                states_d[blk * ROW_BLOCK : (blk + 1) * ROW_BLOCK]
                .rearrange("r (p q) s -> p r q s", p=P),
            )
            # ---- v = relu((x + 3) * 16/3) ; clamp above; floor via int trunc
            v = pool.tile([P, ROW_BLOCK * QP * 3], f32, tag="v")
            nc.scalar.activation(v[:], st[:], Act.Relu, bias=bias16[:], scale=SCALE)
            nc.vector.tensor_scalar(v[:], v[:], 31.4, None, op0=Alu.min)
            iv = pool.tile([P, ROW_BLOCK * QP * 3], i32, tag="iv")
            nc.vector.tensor_copy(iv[:], v[:])  # HW rounds; v = true-0.5 -> floor
            # ---- lin = i0 + 32*i1 + 1024*i2 ; hi/lo split
            v3 = iv[:].rearrange("p (rq s) -> p rq s", s=3)
            lin = pool.tile([P, ROW_BLOCK * QP], f32, tag="lin")
            nc.vector.scalar_tensor_tensor(
                lin[:], in0=v3[:, :, 1], scalar=32.0, in1=v3[:, :, 0],
                op0=Alu.mult, op1=Alu.add,
            )
            nc.vector.scalar_tensor_tensor(
                lin[:], in0=v3[:, :, 2], scalar=1024.0, in1=lin[:],
                op0=Alu.mult, op1=Alu.add,
            )
            s_ = pool.tile([P, ROW_BLOCK * QP], f32, tag="s_")
            nc.vector.tensor_scalar(
                s_[:], lin[:], 1.0 / 256.0, -0.5, op0=Alu.mult, op1=Alu.add
            )
            hi_i = pool.tile([P, ROW_BLOCK * QP], i32, tag="hi_i")
            nc.vector.tensor_copy(hi_i[:], s_[:])  # round(lin/256 - 0.5) = lin >> 8
            hi = pool.tile([P, ROW_BLOCK * QP], f32, tag="hi")
            nc.vector.tensor_copy(hi[:], hi_i[:])
            lo = pool.tile([P, ROW_BLOCK * QP], f32, tag="lo")
            nc.vector.scalar_tensor_tensor(
                lo[:], in0=hi_i[:], scalar=-256.0, in1=lin[:], op0=Alu.mult, op1=Alu.add
            )
            if ACT_NUM:
                nhi = pool.tile([P, ROW_BLOCK * QP], f32, tag="nhi")
                nc.vector.tensor_scalar(nhi[:], hi[:], -1.0, None, op0=Alu.mult)
                nlo = pool.tile([P, ROW_BLOCK * QP], f32, tag="nlo")
                nc.vector.tensor_scalar(nlo[:], lo[:], -1.0, None, op0=Alu.mult)

            # ---- per row: one-hot + matmul accumulate, then scale + store
            outt = pool.tile([P, ROW_BLOCK * W_LO], f32, tag="outt")
            for rr in range(ROW_BLOCK):
                acc = psum.tile([P, W_LO], f32, tag="acc")
                for c in range(QP):
                    cc = rr * QP + c
                    on_act = (c * ACT_NUM) % 8 < ACT_NUM
                    ohi = opool.tile([P, W_HI], bf16, tag="ohi")
                    olo = opool.tile([P, W_LO], bf16, tag="olo")
                    if on_act:
                        # one-hot = relu(1 - (iota - idx)^2), exact on integers
                        sq = opool.tile([P, W_HI + W_LO], bf16, tag="sq")
                        nc.scalar.activation(
                            sq[:, :W_HI], iota_t[:, :W_HI], Act.Square,
                            bias=nhi[:, cc : cc + 1], scale=1.0,
                        )
                        nc.scalar.activation(
                            sq[:, W_HI:], iota_t[:], Act.Square,
                            bias=nlo[:, cc : cc + 1], scale=1.0,
                        )
                        nc.scalar.activation(
                            ohi[:], sq[:, :W_HI], Act.Relu, bias=one_c[:], scale=-1.0
                        )
                        nc.scalar.activation(
                            olo[:], sq[:, W_HI:], Act.Relu, bias=one_c[:], scale=-1.0
                        )
                    else:
                        hi_eng = (
                            nc.gpsimd
                            if (c * GPSIMD_NUM) % GPSIMD_DEN < GPSIMD_NUM
                            else nc.vector
                        )
                        hi_eng.tensor_scalar(
                            ohi[:], iota_t[:, :W_HI], hi[:, cc : cc + 1], None,
                            op0=Alu.is_equal,
                        )
                        lo_eng = (
                            nc.gpsimd
                            if ((c + 4) * GPSIMD_LO_NUM) % GPSIMD_DEN < GPSIMD_LO_NUM
                            else nc.vector
                        )
                        lo_eng.tensor_scalar(
                            olo[:], iota_t[:], lo[:, cc : cc + 1], None,
                            op0=Alu.is_equal,
                        )
                    nc.tensor.matmul(
                        acc[:], ohi[:], olo[:], start=(c == 0), stop=(c == QP - 1)
                    )
                nc.scalar.mul(
                    outt[:, rr * W_LO : (rr + 1) * W_LO], acc[:], 1.0 / NPTS
                )
            nc.sync.dma_start(
                out_d[blk * ROW_BLOCK : (blk + 1) * ROW_BLOCK]
                .rearrange("r (h l) -> h r l", h=P),
                outt[:].rearrange("p (r l) -> p r l", r=ROW_BLOCK),
            )
    nc.finalize()
    if legalize:
        _legalize_drain_waits(nc)
    return nc


def kernel(states, edges=None, bins=None, **_ignored):
    global last_results
    from concourse.bass_utils import run_bass_kernel_spmd
    import ml_dtypes

    states = np.ascontiguousarray(np.asarray(states), dtype=np.float32)
    assert states.shape == (B_FULL, NPTS, 3), states.shape

    if "nc" not in _CACHE:
        _CACHE["nc"] = _build_program()
    nc = _CACHE["nc"]

    iota_np = np.broadcast_to(
        np.arange(W_LO, dtype=np.float32).astype(ml_dtypes.bfloat16), (P, W_LO)
    ).copy()

    in_maps = []
    for i in range(N_CORES):
        in_maps.append(
            {
                "states": np.ascontiguousarray(states[i * ROWS : (i + 1) * ROWS]),
                "iota": iota_np,
            }
        )

    trace = os.environ.get("BASS_HISTO_TRACE", "0") == "1"
    res = run_bass_kernel_spmd(nc, in_maps, list(range(N_CORES)), trace=trace)
    last_results = res

    out = np.concatenate([res.results[i]["out"] for i in range(N_CORES)], axis=0)
    return out.astype(np.float32)



# revision 7
# speedup vs baseline: 1.3746x; 1.3746x over previous
"""Trainium2 Bass kernel: per-row 3D histogram binning (BinDensityEncoder).

states [512, 8192, 3] f32 -> per-row joint histogram over 32^3 = 32768 bins,
normalized by N=8192.  Data-parallel over batch rows: 8 cores x 64 rows.

Algorithm (per core, per batch row):
  - bin index per dim: i = clamp(floor((x+3) * 16/3), 0, 31)   (uniform edges)
  - linear index: lin = i0 + 32*i1 + 1024*i2  in [0, 32768)
  - split lin = hi*256 + lo  (hi in [0,128), lo in [0,256))
  - one-hot(hi) [128pts, 128] and one-hot(lo) [128pts, 256] in bf16 via
    iota-compare (tensor_scalar is_equal, 4x DVE mode), then TensorE matmul
    one-hot(hi)^T @ one-hot(lo) accumulated in PSUM over the row's 64
    point-chunks -> [128, 256] = the row histogram.

Measured op economics (HW microbench):
  - DVE tensor_scalar (tile scalar) = ~131ns fixed + 0.26ns/elem (4x mode):
    ohi FD128 = 164ns, olo FD256 = 197ns -> 361ns/chunk on DVE.
  - ACT activation = (FD+352)/1.2 ns; Square+Exp one-hot = ~1.8us/chunk but
    runs on the otherwise-idle Scalar engine -> offload ACT_NUM/8 of chunks.
  - PE matmul N=256 + LDWEIGHTS = ~113ns/chunk when fed back-to-back.
"""

import os
import sys

import numpy as np

for _p in ("/opt/trn_rl_repo", "/root/.axon_site/_ro/trn_rl_repo"):
    if _p not in sys.path and os.path.isdir(_p):
        sys.path.insert(0, _p)

P = 128
N_CORES = 8
B_FULL = 512
ROWS = B_FULL // N_CORES  # 64 rows per core
NPTS = 8192
QP = NPTS // P  # 64 points per partition (also: chunks per row)
W_HI = 128
W_LO = 256
OUT_DIM = 32768
SCALE = 16.0 / 3.0  # 1 / bin_width, bin_width = 6/32
ROW_BLOCK = 8  # rows per input/output DMA batch

# Fraction of chunks whose one-hots are built on the scalar engine (ACT)
# via exp(-30 * (iota - idx)^2) (Square then Exp, one table set) = ACT_NUM/8.
ACT_NUM = int(os.environ.get("HISTO_ACT_NUM", "3"))
# Which one-hot(s) the ACT path builds for offloaded chunks: "hi", "lo", "both".
# Offloading only ohi is the LP optimum: saves 164ns DVE per 800ns ACT.
ACT_MODE = os.environ.get("HISTO_ACT_MODE", "hi")
OH_BUFS = int(os.environ.get("HISTO_OH_BUFS", "6"))
MAIN_BUFS = int(os.environ.get("HISTO_MAIN_BUFS", "2"))
PSUM_BUFS = int(os.environ.get("HISTO_PSUM_BUFS", "2"))

_CACHE = {}

# Set by the most recent kernel() call when BASS_HISTO_TRACE=1; test.py reads it.
last_results = None


def _legalize_drain_waits(nc, max_waits=1):
    """This image's walrus allows only one sync-wait per instruction.
    Move extra waits onto inserted NoOps just before (same engine, so
    per-engine program order preserves the wait semantics)."""
    import concourse.mybir as mybir

    for fn in nc.m.functions:
        for bb in fn.blocks:
            new_list = []
            changed = False
            for ins in bb.instructions:
                si = ins.sync_info
                waits = list(si.on_wait) if si else []
                if len(waits) > max_waits:
                    kept = waits[:max_waits]
                    extra = waits[max_waits:]
                    for k, w in enumerate(extra):
                        new_list.append(
                            mybir.InstNoOp(
                                name=f"{ins.name}-w{k}",
                                engine=ins.engine,
                                ins=[],
                                outs=[],
                                sync_info=mybir.SyncInfo(on_update=[], on_wait=[w]),
                            )
                        )
                    si.on_wait = kept
                    changed = True
                new_list.append(ins)
            if changed:
                bb.instructions = new_list
    return nc


def _build_program():
    import concourse.bass as bass
    import concourse.mybir as mybir
    import concourse.tile as tile
    from contextlib import ExitStack

    f32 = mybir.dt.float32
    i32 = mybir.dt.int32
    bf16 = mybir.dt.bfloat16
    Alu = mybir.AluOpType
    Act = mybir.ActivationFunctionType

    nc = bass.Bass()
    states_d = nc.declare_dram_parameter("states", [ROWS, NPTS, 3], f32, isOutput=False)
    iota_d = nc.declare_dram_parameter("iota", [P, W_LO], bf16, isOutput=False)
    out_d = nc.declare_dram_parameter("out", [ROWS, OUT_DIM], f32, isOutput=True)

    with tile.TileContext(nc) as tc, ExitStack() as ctx:
        const_pool = ctx.enter_context(tc.tile_pool(name="const", bufs=1))
        pool = ctx.enter_context(tc.tile_pool(name="main", bufs=MAIN_BUFS))
        opool = ctx.enter_context(tc.tile_pool(name="oh", bufs=OH_BUFS))
        psum = ctx.enter_context(
            tc.tile_pool(name="psum", bufs=PSUM_BUFS, space="PSUM")
        )

        iota_t = const_pool.tile([P, W_LO], bf16)
        nc.sync.dma_start(iota_t[:], iota_d[:])
        # bias = 16 - 0.5: HW f32->int32 convert rounds to nearest, so compute
        # v = true_v - 0.5 and round to get floor(true_v).
        bias16 = const_pool.tile([P, 1], f32)
        nc.gpsimd.memset(bias16[:], 15.5)
        biasm05 = const_pool.tile([P, 1], f32)
        nc.gpsimd.memset(biasm05[:], -0.5)

        n_blocks = ROWS // ROW_BLOCK
        for blk in range(n_blocks):
            # ---- load ROW_BLOCK rows: [128, ROW_BLOCK * QP * 3] f32
            st = pool.tile([P, ROW_BLOCK * QP * 3], f32, tag="st")
            nc.sync.dma_start(
                st[:].rearrange("p (r q s) -> p r q s", r=ROW_BLOCK, s=3),
                states_d[blk * ROW_BLOCK : (blk + 1) * ROW_BLOCK]
                .rearrange("r (p q) s -> p r q s", p=P),
            )
            # ---- v = relu((x + 3) * 16/3) ; clamp above; floor via int trunc
            v = pool.tile([P, ROW_BLOCK * QP * 3], f32, tag="v")
            nc.scalar.activation(v[:], st[:], Act.Relu, bias=bias16[:], scale=SCALE)
            nc.vector.tensor_scalar(v[:], v[:], 31.4, None, op0=Alu.min)
            iv = pool.tile([P, ROW_BLOCK * QP * 3], i32, tag="iv")
            nc.vector.tensor_copy(iv[:], v[:])  # HW rounds; v = true-0.5 -> floor
            # ---- lin = i0 + 32*i1 + 1024*i2 ; hi/lo split
            v3 = iv[:].rearrange("p (rq s) -> p rq s", s=3)
            lin = pool.tile([P, ROW_BLOCK * QP], f32, tag="lin")
            nc.vector.scalar_tensor_tensor(
                lin[:], in0=v3[:, :, 1], scalar=32.0, in1=v3[:, :, 0],
                op0=Alu.mult, op1=Alu.add,
            )
            nc.vector.scalar_tensor_tensor(
                lin[:], in0=v3[:, :, 2], scalar=1024.0, in1=lin[:],
                op0=Alu.mult, op1=Alu.add,
            )
            # s_ = lin/256 - 0.5 on ACT (Relu safe: lin >= 0)
            s_ = pool.tile([P, ROW_BLOCK * QP], f32, tag="s_")
            nc.scalar.activation(s_[:], lin[:], Act.Relu, bias=biasm05[:], scale=1.0 / 256.0)
            hi_i = pool.tile([P, ROW_BLOCK * QP], i32, tag="hi_i")
            nc.vector.tensor_copy(hi_i[:], s_[:])  # round(lin/256 - 0.5) = lin >> 8
            hi = pool.tile([P, ROW_BLOCK * QP], f32, tag="hi")
            nc.vector.tensor_copy(hi[:], hi_i[:])
            lo = pool.tile([P, ROW_BLOCK * QP], f32, tag="lo")
            nc.vector.scalar_tensor_tensor(
                lo[:], in0=hi_i[:], scalar=-256.0, in1=lin[:], op0=Alu.mult, op1=Alu.add
            )
            if ACT_NUM and ACT_MODE in ("hi", "both"):
                nhi = pool.tile([P, ROW_BLOCK * QP], f32, tag="nhi")
                nc.vector.tensor_scalar(nhi[:], hi[:], -1.0, None, op0=Alu.mult)
            if ACT_NUM and ACT_MODE in ("lo", "both"):
                nlo = pool.tile([P, ROW_BLOCK * QP], f32, tag="nlo")
                nc.vector.tensor_scalar(nlo[:], lo[:], -1.0, None, op0=Alu.mult)

            # ---- per row: one-hot + matmul accumulate, then scale + store
            outt = pool.tile([P, ROW_BLOCK * W_LO], f32, tag="outt")
            for rr in range(ROW_BLOCK):
                acc = psum.tile([P, W_LO], f32, tag="acc")
                for c in range(QP):
                    cc = rr * QP + c
                    on_act = (c * ACT_NUM) % 8 < ACT_NUM
                    act_hi = on_act and ACT_MODE in ("hi", "both")
                    act_lo = on_act and ACT_MODE in ("lo", "both")
                    ohi = opool.tile([P, W_HI], bf16, tag="ohi")
                    olo = opool.tile([P, W_LO], bf16, tag="olo")
                    if act_hi:
                        # one-hot = exp(-30 * (iota - idx)^2): exact 1 at match,
                        # <1e-13 elsewhere. Square and Exp share one table set.
                        sqh = opool.tile([P, W_HI], f32, tag="sqh")
                        nc.scalar.activation(
                            sqh[:], iota_t[:, :W_HI], Act.Square,
                            bias=nhi[:, cc : cc + 1], scale=1.0,
                        )
                        nc.scalar.activation(
                            ohi[:], sqh[:], Act.Exp, bias=0.0, scale=-30.0
                        )
                    else:
                        nc.vector.tensor_scalar(
                            ohi[:], iota_t[:, :W_HI], hi[:, cc : cc + 1], None,
                            op0=Alu.is_equal,
                        )
                    if act_lo:
                        sql = opool.tile([P, W_LO], f32, tag="sql")
                        nc.scalar.activation(
                            sql[:], iota_t[:], Act.Square,
                            bias=nlo[:, cc : cc + 1], scale=1.0,
                        )
                        nc.scalar.activation(
                            olo[:], sql[:], Act.Exp, bias=0.0, scale=-30.0
                        )
                    else:
                        nc.vector.tensor_scalar(
                            olo[:], iota_t[:], lo[:, cc : cc + 1], None,
                            op0=Alu.is_equal,
                        )
                    nc.tensor.matmul(
                        acc[:], ohi[:], olo[:], start=(c == 0), stop=(c == QP - 1)
                    )
                nc.scalar.mul(
                    outt[:, rr * W_LO : (rr + 1) * W_LO], acc[:], 1.0 / NPTS
                )
            nc.sync.dma_start(
                out_d[blk * ROW_BLOCK : (blk + 1) * ROW_BLOCK]
                .rearrange("r (h l) -> h r l", h=P),
                outt[:].rearrange("p (r l) -> p r l", r=ROW_BLOCK),
            )
    nc.finalize()
    _legalize_drain_waits(nc)
    return nc


def kernel(states, edges=None, bins=None, **_ignored):
    global last_results
    from concourse.bass_utils import run_bass_kernel_spmd
    import ml_dtypes

    states = np.ascontiguousarray(np.asarray(states), dtype=np.float32)
    assert states.shape == (B_FULL, NPTS, 3), states.shape

    if "nc" not in _CACHE:
        _CACHE["nc"] = _build_program()
    nc = _CACHE["nc"]

    iota_np = np.broadcast_to(
        np.arange(W_LO, dtype=np.float32).astype(ml_dtypes.bfloat16), (P, W_LO)
    ).copy()

    in_maps = []
    for i in range(N_CORES):
        in_maps.append(
            {
                "states": np.ascontiguousarray(states[i * ROWS : (i + 1) * ROWS]),
                "iota": iota_np,
            }
        )

    trace = os.environ.get("BASS_HISTO_TRACE", "0") == "1"
    res = run_bass_kernel_spmd(nc, in_maps, list(range(N_CORES)), trace=trace)
    last_results = res

    out = np.concatenate([res.results[i]["out"] for i in range(N_CORES)], axis=0)
    return out.astype(np.float32)
